# revision 51
# baseline (speedup 1.0000x reference)
"""NeuroSAT message-passing GNN on 8 TRN2 NeuronCores (Bass/Tile).

Sharding: clause dim sharded 8-way (2048 padded clauses/core); literal dim
permuted so core i owns problem i's 500 vars (+12 pads) as 1024 lit rows
(512 pos + 512 neg).  Per round:
  GEMM1 LC.T = L_pre.T @ B1 (k over 8192 AG-gathered lits; B1 mostly
  resident in SBUF) ; C-LSTM + C_pre MLP per 512-clause chunk, each chunk's
  fp8 k-tiles AllGathered while the next chunk computes ; GEMM2 accumulates
  CL for the core's OWN 1024 lits over all 16384 clauses directly in psum
  (no ReduceScatter) ; L-LSTM + L_pre MLP per half ; AllGather L_pre fp8.
M (counts) is exact in fp8e4m3; fp8 M blocks are the moving operand against
fp8 stationary activations (DoubleRow).
"""

import numpy as np
import ml_dtypes

import concourse.bass as bass
import concourse.bacc as bacc
import concourse.mybir as mybir
import concourse.tile as tile
from concourse import bass_utils

F32 = mybir.dt.float32
BF16 = mybir.dt.bfloat16
FP8 = mybir.dt.float8e4
AF = mybir.ActivationFunctionType

N_CORES = 8
DIM = 128
N_ROUNDS = 16
N_VARS = 4000
VPC = 500            # real vars per core (= vars per problem)
VPAD = 512           # padded vars per core
LL = 2 * VPAD        # 1024 lit rows per core
LPAD = N_CORES * LL  # 8192
CC = 2048            # padded clauses per core
CPAD = N_CORES * CC  # 16384
KL = LPAD // 128     # 64 k-tiles over lits
KC = CC // 128       # 16 k-tiles over clauses

B1_RES_SET = (0, 1, 2, 4, 6, 8, 10, 12, 14)  # resident b1 groups (interleaved
                                              # so streamed-group deadlines
                                              # spread across GEMM1)
N_WARM_G1 = 30        # dummy MM prefix on GEMM1 (bridge AG0 wait)
N_WARM_L = 0
G2_WARMS = (24, 22, 20, 16)  # warm MMs bridging each cprx AllGather         # scratch warm MMs between early/late L-gate groups

# GEMM2 groups: group g computes 512-lit chunks J_SETS[g]; chunk j covers
# local lit rows [512*(j%2)...) of destination core j//2.  Groups 0,1 cover
# all even j (a2a half 0 = every core's rows 0:512); groups 2,3 odd j.
J_SETS = [[0, 2, 4, 6], [8, 10, 12, 14], [1, 3, 5, 7], [9, 11, 13, 15]]

nbf = ml_dtypes.bfloat16
nf8 = ml_dtypes.float8_e4m3

_CACHE = {}


def _build():
    """Build + compile the SPMD program once (shape-only, no input values)."""
    if "nc" in _CACHE:
        return _CACHE["nc"]

    nc = bacc.Bacc("TRN2", target_bir_lowering=False, debug=False,
                   num_devices=N_CORES)

    def din(name, shape, dt):
        return nc.dram_tensor(name, shape, dt, kind="ExternalInput")

    # b1: 16 packed groups of 4 k-tiles; rows ordered [half h, core c, r<512]
    b1 = din("b1", [KL // 4, DIM, 4 * CC], FP8)
    # b2[q][c]: M.T[clauses of core c quarter q, own 1024 lits] as 4
    # k-tiles packed per DMA: [4 quarters, 8 cores, 128, 4*1024]
    b2 = din("b2", [4, N_CORES, DIM, 4 * 1024], FP8)
    lh0t = din("lh0t", [DIM, LL], BF16)
    ch0t = din("ch0t", [DIM, CC], BF16)
    id128 = din("id128", [DIM, DIM], BF16)

    w = {}
    for p in ("lmsg", "cmsg", "lvote"):
        for i in (1, 2, 3):
            shp = [DIM, 1] if (p == "lvote" and i == 3) else [DIM, DIM]
            w[f"{p}_w{i}t"] = din(f"{p}_w{i}t", shp, BF16)
            bshp = [1, 1] if (p == "lvote" and i == 3) else [DIM, 1]
            w[f"{p}_b{i}"] = din(f"{p}_b{i}", bshp, F32)
    w["cu_wiht"] = din("cu_wiht", [DIM, 4 * DIM], BF16)
    w["cu_whht"] = din("cu_whht", [DIM, 4 * DIM], BF16)
    w["lu_wiht_cl"] = din("lu_wiht_cl", [DIM, 4 * DIM], BF16)
    w["lu_wiht_fl"] = din("lu_wiht_fl", [DIM, 4 * DIM], BF16)
    w["lu_whht"] = din("lu_whht", [DIM, 4 * DIM], BF16)
    cu_bias_d = din("cu_bias", [4, DIM], F32)
    lu_bias_d = din("lu_bias", [4, DIM], F32)

    vote_out = nc.dram_tensor("vote", [1, LL], F32, kind="ExternalOutput")

    with tile.TileContext(nc) as tc, \
         tc.tile_pool(name="const", bufs=1) as const, \
         tc.tile_pool(name="sb", bufs=2) as sb, \
         tc.tile_pool(name="sb3", bufs=2) as sb3, \
         tc.tile_pool(name="ps", bufs=4, space="PSUM") as ps, \
         tc.tile_pool(name="pstr", bufs=2, space="PSUM") as pstr, \
         tc.tile_pool(name="dram", bufs=2, space="DRAM") as dram:

        # ---- load constants/weights into SBUF
        cw = {}
        for k in w:
            t = const.tile(list(w[k].shape), w[k].dtype, tag=f"cw_{k}")
            nc.sync.dma_start(t[:], w[k].ap())
            cw[k] = t
        for k, dte in (("cu_bias", cu_bias_d), ("lu_bias", lu_bias_d)):
            t = const.tile([DIM, 4], F32, tag=f"cw_{k}")
            nc.sync.dma_start(t[:], dte.ap().rearrange("g p -> p g"))
            cw[k] = t
        idt = const.tile([DIM, DIM], BF16, tag="idt")
        nc.sync.dma_start(idt[:], id128.ap())
        zbf = const.tile([DIM, 512], BF16, tag="zbf")
        nc.vector.memset(zbf[:], 0.0)

        # ---- resident b1 groups
        b1r = {}
        for g in B1_RES_SET:
            t = const.tile([DIM, 4 * CC], FP8, tag=f"b1r{g}")
            nc.sync.dma_start(t[0:64, :], b1.ap()[g, 0:64, :])
            nc.scalar.dma_start(t[64:DIM, :], b1.ap()[g, 64:DIM, :])
            b1r[g] = t

        # ---- persistent state (feature-major)
        lht = const.tile([DIM, LL], BF16, tag="lht")
        lct = const.tile([DIM, LL], BF16, tag="lct")
        cht = const.tile([DIM, CC], BF16, tag="cht")
        cct = const.tile([DIM, CC], BF16, tag="cct")
        nc.sync.dma_start(lht[:], lh0t.ap())
        nc.sync.dma_start(cht[:], ch0t.ap())
        nc.vector.memset(lct[:], 0.0)
        nc.vector.memset(cct[:], 0.0)

        def dma2(dst, src):
            """Bulk stream transfer on the Sync HWDGE queue only; the
            Activation queue is reserved for latency-critical staging so
            stream pushes never head-block compute activations."""
            nc.sync.dma_start(dst[:, :], src[:, :])

        def mlp_chunk(x, pfx, sl, n, out_dt=BF16, tagsfx=""):
            """3-layer MLP on columns sl (chunks of <=512) of x [128, *]."""
            cur = x
            for li in (1, 2, 3):
                wt = cw[f"{pfx}_w{li}t"]
                bt = cw[f"{pfx}_b{li}"]
                m = wt.shape[1]
                o = sb.tile([m, n], out_dt if li == 3 else BF16,
                            tag=f"{pfx}_h{li}{tagsfx}", name=f"{pfx}_h{li}{tagsfx}")
                for rc in range(n // 512):
                    c0 = rc * 512
                    pt = ps.tile([m, 512], F32, tag="ps", name="mlp_ps")
                    src = cur[:, sl.start + c0:sl.start + c0 + 512] if li == 1 \
                        else cur[:, c0:c0 + 512]
                    nc.tensor.matmul(pt[:], wt[:], src, start=True, stop=True)
                    func = AF.Relu if li < 3 else AF.Identity
                    nc.scalar.activation(o[:, c0:c0 + 512], pt[:], func,
                                         bias=bt[:, 0:1])
                cur = o
            return cur

        def lstm_elementwise(gps, bias, c_st, h_st, rc0, n):
            """gps: 4 psum tiles [128, n] (i,f,g,o); updates states [:, rc0:rc0+n]."""
            sl = slice(rc0, rc0 + n)
            sig_i = sb.tile([DIM, n], BF16, tag="lw_si", bufs=1, name="sig_i")
            sig_f = sb.tile([DIM, n], BF16, tag="lw_sf", bufs=1, name="sig_f")
            tng = sb.tile([DIM, n], BF16, tag="lw_tg", bufs=1, name="tng")
            sig_o = sb.tile([DIM, n], BF16, tag="lw_so", bufs=1, name="sig_o")
            nc.scalar.activation(sig_i[:], gps[0][:], AF.Sigmoid, bias=bias[:, 0:1])
            nc.scalar.activation(sig_f[:], gps[1][:], AF.Sigmoid, bias=bias[:, 1:2])
            nc.scalar.activation(tng[:], gps[2][:], AF.Tanh, bias=bias[:, 2:3])
            nc.scalar.activation(sig_o[:], gps[3][:], AF.Sigmoid, bias=bias[:, 3:4])
            t1 = sb.tile([DIM, n], BF16, tag="lw_t1", bufs=1, name="t1")
            nc.vector.tensor_mul(t1[:], sig_f[:], c_st[:, sl])
            t2 = sb.tile([DIM, n], BF16, tag="lw_t2", bufs=1, name="t2")
            nc.vector.tensor_mul(t2[:], sig_i[:], tng[:])
            nc.vector.tensor_add(c_st[:, sl], t1[:], t2[:])
            tnc = sb.tile([DIM, n], BF16, tag="lw_tc", bufs=1, name="tnc")
            nc.scalar.activation(tnc[:], c_st[:, sl], AF.Tanh)
            nc.vector.tensor_mul(h_st[:, sl], sig_o[:], tnc[:])

        def c_lstm_chunk(lc, rc):
            """C-LSTM chunk rc with 2-wave gates (psum tag-ps peak = 2)."""
            sl = slice(rc * 512, (rc + 1) * 512)
            bias = cw["cu_bias"]
            gi = ps.tile([DIM, 512], F32, tag="ps", name="cgi")
            gf = ps.tile([DIM, 512], F32, tag="ps", name="cgf")
            for gp, g in ((gi, 0), (gf, 1)):
                gsl = slice(g * DIM, (g + 1) * DIM)
                nc.tensor.matmul(gp[:], cw["cu_wiht"][:, gsl], lc[:],
                                 start=True, stop=False)
                nc.tensor.matmul(gp[:], cw["cu_whht"][:, gsl], cht[:, sl],
                                 start=False, stop=True)
            sig_i = sb.tile([DIM, 512], BF16, tag="lw_si", bufs=1, name="sig_i")
            sig_f = sb.tile([DIM, 512], BF16, tag="lw_sf", bufs=1, name="sig_f")
            nc.scalar.activation(sig_i[:], gi[:], AF.Sigmoid, bias=bias[:, 0:1])
            nc.scalar.activation(sig_f[:], gf[:], AF.Sigmoid, bias=bias[:, 1:2])
            t1 = sb.tile([DIM, 512], BF16, tag="lw_t1", bufs=1, name="t1")
            nc.vector.tensor_mul(t1[:], sig_f[:], cct[:, sl])
            gg = ps.tile([DIM, 512], F32, tag="ps", name="cgg")
            go = ps.tile([DIM, 512], F32, tag="ps", name="cgo")
            for gp, g in ((gg, 2), (go, 3)):
                gsl = slice(g * DIM, (g + 1) * DIM)
                nc.tensor.matmul(gp[:], cw["cu_wiht"][:, gsl], lc[:],
                                 start=True, stop=False)
                nc.tensor.matmul(gp[:], cw["cu_whht"][:, gsl], cht[:, sl],
                                 start=False, stop=True)
            tng = sb.tile([DIM, 512], BF16, tag="lw_tg", bufs=1, name="tng")
            sig_o = sb.tile([DIM, 512], BF16, tag="lw_so", bufs=1, name="sig_o")
            nc.scalar.activation(tng[:], gg[:], AF.Tanh, bias=bias[:, 2:3])
            nc.scalar.activation(sig_o[:], go[:], AF.Sigmoid, bias=bias[:, 3:4])
            t2 = sb.tile([DIM, 512], BF16, tag="lw_t2", bufs=1, name="t2")
            nc.vector.tensor_mul(t2[:], sig_i[:], tng[:])
            nc.vector.tensor_add(cct[:, sl], t1[:], t2[:])
            tnc = sb.tile([DIM, 512], BF16, tag="lw_tc", bufs=1, name="tnc")
            nc.scalar.activation(tnc[:], cct[:, sl], AF.Tanh)
            nc.vector.tensor_mul(cht[:, sl], sig_o[:], tnc[:])

        def c_gemm2(lct_ps, r):
            """C-LSTM + C_pre MLP per 512-clause chunk; each chunk's fp8
            k-tiles are AllGathered while the next chunk computes; GEMM2
            accumulates CL for the core's own 1024 lits over all 16384
            clauses into two persistent psum accumulators."""
            lc_sb = []
            for rc in range(4):
                t = sb.tile([DIM, 512], BF16, tag="lc_sb", bufs=4,
                            name=f"lc_sb{rc}")
                nc.vector.tensor_copy(t[:], lct_ps[rc][:])
                lc_sb.append(t)
            acc = [ps.tile([DIM, 512], F32, tag="psacc", bufs=2,
                           name=f"cl_acc{j}") for j in range(2)]
            cprx = sb.tile([DIM, 4 * 32 * DIM], FP8, tag="cprx", bufs=1,
                           name="cprx")

            def gemm2_quarter(q, n_warm=0):
                for wi in range(n_warm):
                    wt = ps.tile([DIM, DIM], F32, tag="ps", name="g2warm")
                    nc.tensor.matmul(wt[:], idt[:], zbf[:, 0:DIM],
                                     start=True, stop=True)
                for c in range(N_CORES):
                    b2t = sb3.tile([DIM, 4 * 1024], FP8, tag="b2t", bufs=5,
                                   name="b2t")
                    dma2(b2t[:], b2.ap()[q, c, :, :])
                    v = b2t[:].rearrange("p (t l) -> p t l", l=1024)
                    for dr in range(2):
                        base = (q * 32 + c * 4 + dr * 2) * DIM
                        ck = cprx[:, base:base + 2 * DIM].rearrange(
                            "p (j d) -> p j d", j=2)
                        for j in range(2):
                            nc.tensor.matmul(
                                acc[j][:], ck,
                                v[:, dr * 2:dr * 2 + 2, j * 512:(j + 1) * 512],
                                start=(q == 0 and c == 0 and dr == 0),
                                stop=(q == 3 and c == N_CORES - 1 and dr == 1),
                                perf_mode=mybir.MatmulPerfMode.DoubleRow)

            for q in range(4):
                sl = slice(q * 512, (q + 1) * 512)
                c_lstm_chunk(lc_sb[q], q)
                cpreT = mlp_chunk(cht, "cmsg", sl, 512)
                agc_in = dram.tile([512, DIM], FP8, tag=f"agc_in{q}",
                                   name=f"agc_in{q}_{r}")
                for t in range(4):
                    pt = pstr.tile([DIM, DIM], BF16, tag="pstr", name="cp_tr")
                    nc.tensor.transpose(pt[:], cpreT[:, t * DIM:(t + 1) * DIM],
                                        idt[:])
                    st = sb.tile([DIM, DIM], FP8, tag="tr_st", bufs=4,
                                 name="cp_st")
                    nc.vector.tensor_copy(st[:], pt[:])
                    nc.scalar.dma_start(agc_in[t * DIM:(t + 1) * DIM, :], st[:])
                agc_out = dram.tile([4096, DIM], FP8, tag=f"agc_out{q}",
                                    name=f"agc_out{q}_{r}")
                collective("AllGather", mybir.AluOpType.bypass,
                           agc_in, agc_out)
                base = q * 32 * DIM
                s3 = agc_out[:].rearrange("(t p) d -> p t d", p=DIM)
                d3 = cprx[:, base:base + 32 * DIM].rearrange(
                    "p (t d) -> p t d", d=DIM)
                nc.gpsimd.dma_start(d3[:], s3[:])
                if q >= 1:
                    gemm2_quarter(q - 1, G2_WARMS[q - 1])
            gemm2_quarter(3, G2_WARMS[3])
            return acc

        def l_half(h, acc, flip_ap, r, ag_in):
            """L-LSTM + L_pre MLP + transposes for local half h; CL comes
            from the local GEMM2 psum accumulator acc[h].  flip_ap holds
            the pre-update opposite-polarity rows; the own-h term reads
            lht directly (not yet updated for this half)."""
            sl = slice(h * 512, (h + 1) * 512)
            clf = sb.tile([DIM, 512], BF16, tag=f"clf{h}", bufs=1,
                          name=f"clf{h}")
            nc.vector.tensor_copy(clf[:], acc[h][:])
            gps = [ps.tile([DIM, 512], F32, tag="ps", name=f"lg{h}_{i}")
                   for i in range(4)]
            for g in range(4):
                gsl = slice(g * DIM, (g + 1) * DIM)
                nc.tensor.matmul(gps[g][:], cw["lu_wiht_cl"][:, gsl],
                                 clf[:], start=True, stop=False)
                nc.tensor.matmul(gps[g][:], cw["lu_wiht_fl"][:, gsl],
                                 flip_ap, start=False, stop=False)
                nc.tensor.matmul(gps[g][:], cw["lu_whht"][:, gsl],
                                 lht[:, sl], start=False, stop=True)
            lstm_elementwise(gps, cw["lu_bias"], lct, lht, h * 512, 512)
            if ag_in is None:
                return
            lpre_h = mlp_chunk(lht, "lmsg", sl, 512, tagsfx=f"_{h}")
            for t in range(4):
                tsl = slice(t * DIM, (t + 1) * DIM)
                pt = pstr.tile([DIM, DIM], BF16, tag="pstr", name="lp_tr")
                nc.tensor.transpose(pt[:], lpre_h[:, tsl], idt[:])
                st = sb.tile([DIM, DIM], FP8, tag="tr_st", bufs=4,
                             name="tr_st")
                nc.vector.tensor_copy(st[:], pt[:])
                nc.scalar.dma_start(ag_in[tsl, :], st[:])

        def gemm1(lpre_sb, n_warm=0):
            """GEMM1: LC.T [128, 2048] psum accums over 64 packed k-tiles."""
            lct_ps = [ps.tile([DIM, 512], F32, tag="ps", name=f"g1_{i}")
                      for i in range(4)]
            for wi in range(n_warm):
                nc.tensor.matmul(lct_ps[wi % 4][:], idt[:], zbf[:],
                                 start=(wi < 4), stop=False)
            for grp in range(KL // 4):
                if grp in b1r:
                    b1v = b1r[grp][:].rearrange("p (t c) -> p t c", c=CC)
                    halves = [(kk, b1v[:, kk:kk + 2, :]) for kk in (0, 2)]
                else:
                    halves = []
                    for kk in (0, 2):
                        b1t = sb3.tile([DIM, 2 * CC], FP8, tag="b1t", bufs=6,
                                       name="b1t")
                        dma2(b1t[:], b1.ap()[grp, :, kk * CC:(kk + 2) * CC])
                        halves.append((kk, b1t[:].rearrange(
                            "p (t c) -> p t c", c=CC)))
                for kk, hv in halves:
                    k = 4 * grp + kk
                    lf = lpre_sb[k // 8]
                    t0 = k % 8
                    lk = lf[:, t0 * DIM:(t0 + 2) * DIM].rearrange(
                        "p (j d) -> p j d", j=2)
                    for c4 in range(4):
                        nc.tensor.matmul(
                            lct_ps[c4][:], lk,
                            hv[:, :, c4 * 512:(c4 + 1) * 512],
                            start=(k == 0 and n_warm == 0),
                            stop=(k == KL - 2),
                            perf_mode=mybir.MatmulPerfMode.DoubleRow)
            return lct_ps

        def load_lpre(ag_outs):
            """Load AG halves as 8 SBUF chunks of 8 k-tiles each."""
            lpre_sb = []
            for c8 in range(8):
                lt = sb.tile([DIM, 8 * DIM], FP8, tag="lpf", bufs=6,
                             name=f"lpf{c8}")
                src = ag_outs[c8 // 4][(c8 % 4) * 1024:(c8 % 4 + 1) * 1024, :]
                s3 = src.rearrange("(t p) d -> p t d", p=DIM)
                d3 = lt[:].rearrange("p (t d) -> p t d", d=DIM)
                nc.gpsimd.dma_start(d3[:], s3[:])
                lpre_sb.append(lt)
            return lpre_sb

        rg = [list(range(N_CORES))]

        def collective(kind, op, cin, cout):
            nc.gpsimd.collective_compute(kind, op, replica_groups=rg,
                                         ins=[cin.opt()], outs=[cout.opt()])

        # ====== round 0 head: L_pre from Lh0 -> ag_in halves ======
        ag_ins = []
        for h in range(2):
            ag_in = dram.tile([512, DIM], FP8, tag=f"ag_in{h}",
                              name=f"ag_in{h}_init")
            lpre_h = mlp_chunk(lht, "lmsg", slice(h * 512, (h + 1) * 512),
                               512, tagsfx=f"_{h}")
            for t in range(4):
                tsl = slice(t * DIM, (t + 1) * DIM)
                pt = pstr.tile([DIM, DIM], BF16, tag="pstr", name="lp_tr0")
                nc.tensor.transpose(pt[:], lpre_h[:, tsl], idt[:])
                st = sb.tile([DIM, DIM], FP8, tag="tr_st", bufs=4,
                             name="tr_st0")
                nc.vector.tensor_copy(st[:], pt[:])
                nc.scalar.dma_start(ag_in[tsl, :], st[:])
            ag_ins.append(ag_in)

        for r in range(N_ROUNDS):
            ag_outs = []
            for h in range(2):
                ag_out = dram.tile([4096, DIM], FP8, tag=f"ag_out{h}",
                                   name=f"ag_out{h}_{r}")
                collective("AllGather", mybir.AluOpType.bypass,
                           ag_ins[h], ag_out)
                ag_outs.append(ag_out)
            lpre_sb = load_lpre(ag_outs)
            lct_ps = gemm1(lpre_sb, N_WARM_G1)
            acc = c_gemm2(lct_ps, r)

            flip0 = sb.tile([DIM, 512], BF16, tag="flip0", bufs=1,
                            name="flip0")
            nc.vector.tensor_copy(flip0[:], lht[:, 0:512])

            last = r == N_ROUNDS - 1
            ag_ins = [None, None]
            if not last:
                ag_ins = [dram.tile([512, DIM], FP8, tag=f"ag_in{h}",
                                    name=f"ag_in{h}_{r}") for h in range(2)]
            l_half(0, acc, lht[:, 512:1024], r, ag_ins[0])
            l_half(1, acc, flip0[:], r, ag_ins[1])

        # ---- vote MLP on final Lh -> [1, 1024] f32
        vt0 = mlp_chunk(lht, "lvote", slice(0, 512), 512, out_dt=F32,
                        tagsfx="_0")
        vt1 = mlp_chunk(lht, "lvote", slice(512, 1024), 512, out_dt=F32,
                        tagsfx="_1")
        nc.sync.dma_start(vote_out.ap()[:, 0:512], vt0[:])
        nc.sync.dma_start(vote_out.ap()[:, 512:1024], vt1[:])

    nc.compile()
    _CACHE["nc"] = nc
    return nc


def _perm_rows(lits):
    """Map global lit index -> permuted row (core-major, 1024 rows/core)."""
    lits = np.asarray(lits)
    neg = lits >= N_VARS
    v = np.where(neg, lits - N_VARS, lits)
    core = v // VPC
    r = v % VPC
    return core * LL + np.where(neg, VPAD + r, r)


def _b1_row_order():
    """B1 rows: [half h, core c, r] -> permuted row c*1024 + h*512 + r."""
    order = np.empty(LPAD, np.int64)
    n = 0
    for h in range(2):
        for c in range(N_CORES):
            order[n:n + 512] = c * LL + h * 512 + np.arange(512)
            n += 512
    return order


def host_prep(inp):
    f32 = np.float32
    idx = inp["L_unpack_indices"].astype(np.int64)
    rows = _perm_rows(idx[:, 0])
    M = np.zeros((LPAD, CPAD), np.float32)
    np.add.at(M, (rows, idx[:, 1]), 1.0)

    row_order = _b1_row_order()
    b1s, b2s = [], []
    for i in range(N_CORES):
        blk = M[:, i * CC:(i + 1) * CC]          # [8192, 2048] permuted rows
        b1o = blk[row_order]                      # AG-concat row order
        # pack 4 k-tiles per DMA group: [16, 128, 4*2048]
        b1p = b1o.reshape(16, 4, DIM, CC).transpose(0, 2, 1, 3) \
                 .reshape(16, DIM, 4 * CC)
        b1s.append(np.ascontiguousarray(b1p).astype(nf8))
        # b2'[q][c] = M[own 1024 lits, clauses of core c quarter q].T as
        # 4 k-tiles of [128 cl, 1024 lit] packed along the free dim
        own = M[i * LL:(i + 1) * LL, :]           # [1024, 16384]
        b2p = np.empty((4, N_CORES, DIM, 4 * 1024), np.float32)
        for q in range(4):
            for c in range(N_CORES):
                cls0 = c * CC + q * 512
                blk2 = own[:, cls0:cls0 + 512].T  # [512 cl, 1024 lit]
                b2p[q, c] = blk2.reshape(4, DIM, 1024).transpose(1, 0, 2) \
                                .reshape(DIM, 4 * 1024)
        b2s.append(np.ascontiguousarray(b2p).astype(nf8))

    def bf(x):
        return np.ascontiguousarray(x).astype(nbf)

    l0 = (inp["L_init_w"][:, 0] + inp["L_init_b"]).astype(f32)
    c0 = (inp["C_init_w"][:, 0] + inp["C_init_b"]).astype(f32)
    common = {
        "lh0t": bf(np.repeat(l0[:, None], LL, axis=1)),
        "ch0t": bf(np.repeat(c0[:, None], CC, axis=1)),
        "id128": bf(np.eye(DIM, dtype=f32)),
        "cu_wiht": bf(inp["Cu_wih"].T), "cu_whht": bf(inp["Cu_whh"].T),
        "lu_wiht_cl": bf(inp["Lu_wih"].T[:DIM]),
        "lu_wiht_fl": bf(inp["Lu_wih"].T[DIM:]),
        "lu_whht": bf(inp["Lu_whh"].T),
        "cu_bias": (inp["Cu_bih"] + inp["Cu_bhh"]).astype(f32).reshape(4, DIM),
        "lu_bias": (inp["Lu_bih"] + inp["Lu_bhh"]).astype(f32).reshape(4, DIM),
    }
    for p, P in (("lmsg", "Lmsg"), ("cmsg", "Cmsg"), ("lvote", "Lvote")):
        for i in (1, 2, 3):
            common[f"{p}_w{i}t"] = bf(inp[f"{P}_w{i}"].T)
            bshape = (1, 1) if (p == "lvote" and i == 3) else (DIM, 1)
            common[f"{p}_b{i}"] = inp[f"{P}_b{i}"].astype(f32).reshape(bshape)
    return [dict(common, b1=b1s[i], b2=b2s[i]) for i in range(N_CORES)]


def kernel(**inputs):
    inp = {k: np.asarray(v) for k, v in inputs.items()}
    in_maps = host_prep(inp)
    nc = _build()
    res = bass_utils.run_bass_kernel_spmd(nc, in_maps,
                                          core_ids=list(range(N_CORES)))
    probs = np.zeros(N_CORES, np.float32)
    for i in range(N_CORES):
        v = res.results[i]["vote"][0]            # [1024]
        s = v[:VPC].astype(np.float64).sum() + \
            v[VPAD:VPAD + VPC].astype(np.float64).sum()
        probs[i] = np.float32(s / (2 * VPC))
    return probs
